# revision 8
# baseline (speedup 1.0000x reference)
"""CRF loss (forward-algorithm partition function minus gold score) on 8 trn2 cores.

Strategy
--------
Data-parallel over batch: 512 sequences -> 64 per core. Inside a core the
T=1024 sequential CRF forward recurrence is parallelized over time using the
Perron-Frobenius contraction of products of positive matrices: the sequence is
split into C=8 chunks that run concurrently as columns of one [48, 512] state
tensor, each chunk re-running the last W=15 steps of its predecessor as warmup
to converge onto the true incoming state direction (measured direction error
~1e-11 after 15 steps). log Z is reassembled from per-chunk log-l1 scales.

The recurrence runs in the exp domain (alpha_t = expT^T alpha . exp(emit_t)),
with a constant e^{-c_abs} absorbed into the transition matrix so magnitudes
stay in fp32/bf16 range without per-step renorm; one exact l1 renorm happens at
the warmup boundary.

Per step: one PE matmul [48x48]@[48,256] per column-group (2 groups for
PE/DVE overlap) + one DVE multiply with the transposed exp(emissions) tile.
Emissions are streamed in "strips" (same local-pair range for all 8 chunks) so
the scan can start after the first strip; each strip is exp'd on ScalarE
(fp32->bf16) and transposed to [label, (chunk, batch)] layout via DMA xbar.

Gold score: emission gather is computed on-device as a one-hot (iota==label)
multiply-accumulate over the full emissions stream (GpSimd); the tiny
labels-only terms (transitions/start/end lookups) and the final mean are
assembled on the host along with the 8-way unshard.
"""

import numpy as np
import ml_dtypes

import concourse.bass as bass
import concourse.bacc as bacc
import concourse.mybir as mybir
from concourse import tile
from concourse.bass_utils import run_bass_kernel_spmd

F32 = mybir.dt.float32
BF16 = mybir.dt.bfloat16
I32 = mybir.dt.int32

NL = 48          # labels
B = 512          # full batch
T = 1024         # sequence length
NCORE = 8
BLOC = B // NCORE  # 64 sequences per core

C = 8            # time chunks (columns groups of the parallel scan)
W = 15           # warmup steps re-run per chunk
LC = 126         # counted steps per chunk  (W + C*LC == T-1)
S = W + LC       # 141 steps executed per chunk column
PLOC = 71        # local t-pairs per chunk: p0 = (1+s)//2 in [0, 70]
CABS = 4.83      # log-growth constant absorbed into exp(trans - CABS)
COLS = C * BLOC  # 512 state columns
EMT = T          # X block (c,q,dt) holds t = 126c+2q+dt; max t = 882+141 = 1023
XFREE = C * PLOC * BLOC   # transposed-emissions tile free size per partition

# io strips: local pair ranges, same for every chunk
STRIPS = [(0, 16), (16, 32), (32, 48), (48, 64), (64, 71)]

assert W + C * LC == T - 1 and LC % 2 == 0

_prog_cache = {}


def _build_program():
    if "nc" in _prog_cache:
        return _prog_cache["nc"]

    nc = bacc.Bacc("TRN2", target_bir_lowering=False, debug=False)

    em = nc.dram_tensor("emissions", [BLOC, EMT, NL], F32, kind="ExternalInput")
    lab = nc.dram_tensor("labels", [BLOC, T], I32, kind="ExternalInput")
    expT = nc.dram_tensor("exp_trans", [NL, NL], BF16, kind="ExternalInput")
    expStart = nc.dram_tensor("exp_start", [NL, 1], F32, kind="ExternalInput")
    expEnd = nc.dram_tensor("exp_end", [NL, 1], BF16, kind="ExternalInput")
    out_scan = nc.dram_tensor("out_scan", [3, COLS], F32, kind="ExternalOutput")
    out_gold = nc.dram_tensor("out_gold", [128, 4], F32, kind="ExternalOutput")

    em_t = em[:].tensor

    with tile.TileContext(nc) as tc:
        with (
            tc.tile_pool(name="big", bufs=1) as big,
            tc.tile_pool(name="strip", bufs=2) as strip_pool,
            tc.tile_pool(name="ebf", bufs=2) as ebf_pool,
            tc.tile_pool(name="goldin", bufs=2) as gold_pool,
            tc.tile_pool(name="oh", bufs=2) as oh_pool,
            tc.tile_pool(name="small", bufs=1) as small,
            tc.tile_pool(name="ps", bufs=2, space="PSUM") as ps_pool,
            tc.tile_pool(name="psfin", bufs=1, space="PSUM") as psfin_pool,
        ):
            # ---- persistent tiles ----
            X = big.tile([128, XFREE], BF16, tag="X")         # transposed exp(em), j padded to 64
            state = big.tile([NL, COLS], BF16, tag="state")
            expT_sb = small.tile([NL, NL], BF16, tag="expT")
            expStart_sb = small.tile([NL, 1], F32, tag="expStart")
            expEnd_sb = small.tile([NL, 1], BF16, tag="expEnd")
            ones_k48 = small.tile([NL, 1], BF16, tag="ones_k48")
            ones_m48 = small.tile([1, NL], F32, tag="ones_m48")
            lab_sb = small.tile([128, T // 2], I32, tag="lab")
            lab16 = small.tile([128, T // 2], mybir.dt.int16, tag="lab16")
            iota_jt = small.tile([128, NL * 128], mybir.dt.int16, tag="iota_jt")
            emitg = small.tile([128, 4], F32, tag="emitg")
            logr = small.tile([1, COLS], F32, tag="logr")
            lw_ones = small.tile([1, COLS], F32, tag="lw_ones")
            lw_end = small.tile([1, COLS], F32, tag="lw_end")
            rinv = small.tile([1, COLS], F32, tag="rinv")

            nc.sync.dma_start(expT_sb[:], expT[:])
            nc.sync.dma_start(expStart_sb[:], expStart[:])
            nc.sync.dma_start(expEnd_sb[:], expEnd[:])
            nc.vector.memset(ones_k48[:], 1.0)
            nc.vector.memset(ones_m48[:], 1.0)
            # labels in half-stacked layout: partition b+64h <- labels[b, 512h+i]
            lab_src = bass.AP(
                tensor=lab[:].tensor, offset=0,
                ap=[[T // 2, 2], [T, BLOC], [1, T // 2]],
            )
            nc.sync.dma_start(lab_sb[:], lab_src)
            nc.vector.tensor_copy(lab16[:], lab_sb[:])
            # iota_jt[p, j, t] = j  (int16, constant along t)
            nc.gpsimd.iota(iota_jt[:].rearrange("p (j t) -> p j t", t=128),
                           pattern=[[1, NL], [0, 128]], base=0,
                           channel_multiplier=0)

            # views of X:  [96, C, PLOC, BLOC]
            Xv = X[:].rearrange("p (c q b) -> p c q b", c=C, b=BLOC)

            # ---- emission streaming: strips of local pairs ----
            def emit_strip(mi):
                q0, q1 = STRIPS[mi]
                nq = q1 - q0
                fsz = nq * 2 * NL  # free size per (chunk-pair) sub-op
                for j0 in range(4):  # chunk pairs (2*j0, 2*j0+1)
                    enat = strip_pool.tile([128, 16 * 2 * NL], F32, tag="enat")
                    ebf = ebf_pool.tile([128, 16 * 2 * 64], BF16, tag="ebf")
                    src = bass.AP(
                        tensor=em_t,
                        offset=(2 * q0 + LC * (2 * j0)) * NL,
                        ap=[[LC * NL, 2], [EMT * NL, BLOC], [NL, nq * 2], [1, NL]],
                    )
                    nc.sync.dma_start(enat[:, 0:fsz], src)
                    en3 = enat[:, 0:fsz].rearrange("p (s j) -> p s j", j=NL)
                    eball = ebf[:, 0:nq * 2 * 64].rearrange(
                        "p (s v) -> p s v", v=64)
                    nc.gpsimd.memset(eball[:, :, NL:64], 0.0)
                    eb3 = eball[:, :, 0:NL]
                    nc.scalar.activation(eb3, en3,
                                         mybir.ActivationFunctionType.Exp)
                    for c2 in range(2):
                        c = 2 * j0 + c2
                        out3d = Xv[:, c, q0:q1, :]
                        nc.sync.dma_start(
                            out3d, ebf[c2 * 64:(c2 + 1) * 64, 0:nq * 2 * 64],
                            transpose=True)

            # ---- gold-score emission gather (one-hot mult-accumulate) ----
            def emit_gold(k):
                halfT = T // 2
                enat2 = gold_pool.tile([128, 128 * NL], F32, tag="goldin")
                oh = oh_pool.tile([128, 128 * NL], BF16, tag="oh")
                src = bass.AP(
                    tensor=em_t, offset=k * 128 * NL,
                    ap=[[halfT * NL, 2], [EMT * NL, BLOC], [NL, 128], [1, NL]],
                )
                nc.sync.dma_start(enat2[:], src)
                # d[p, j, t] = label - j  (2B operands, innermost stride 1 -> 2x)
                d3 = oh[:].rearrange("p (j t) -> p j t", t=128)
                lab_b = lab16[:, k * 128:(k + 1) * 128].unsqueeze(1) \
                    .broadcast_to([128, NL, 128])
                io3 = iota_jt[:].rearrange("p (j t) -> p j t", t=128)
                nc.vector.tensor_tensor(d3, lab_b, io3,
                                        mybir.AluOpType.subtract)
                # emitg[:, k] = sum_t,j (d==0) * em
                em_jt = enat2[:].rearrange("p (t j) -> p t j", j=NL) \
                    .transpose([0, 2, 1])
                nc.vector.scalar_tensor_tensor(
                    d3, d3, 0.0, em_jt,
                    mybir.AluOpType.is_equal, mybir.AluOpType.mult,
                    accum_out=emitg[:, k:k + 1])

            # ---- scan step ----
            def scan_step(s):
                par = (1 + s) % 2
                p0 = (1 + s) // 2
                for g in range(2):
                    ps = ps_pool.tile([NL, COLS // 2], F32, tag=f"ps{g}")
                    gsl = state[:, g * (COLS // 2):(g + 1) * (COLS // 2)]
                    nc.tensor.matmul(ps[:], expT_sb[:], gsl, start=True,
                                     stop=True)
                    xap = X[64 * par:64 * par + 48, :] \
                        .rearrange("p (c q) -> p c q", c=C)[
                            :, 4 * g:4 * g + 4,
                            p0 * BLOC:(p0 + 1) * BLOC]
                    ps3 = ps[:].rearrange("p (c b) -> p c b", b=BLOC)
                    g3 = gsl.rearrange("p (c b) -> p c b", b=BLOC)
                    nc.vector.tensor_tensor(g3, ps3, xap, mybir.AluOpType.mult)

            # ---- emit program ----
            emit_strip(0)
            emit_gold(0)

            # init state
            nc.vector.memset(state[:, BLOC:COLS], 1.0)
            nc.vector.tensor_scalar_mul(state[:, 0:BLOC], X[0:48, 0:BLOC],
                                        expStart_sb[:])

            strip_sched = {8: 1, 40: 2, 72: 3, 104: 4}
            gold_sched = {20: 1, 52: 2, 84: 3}
            for s in range(S):
                if s in strip_sched:
                    emit_strip(strip_sched[s])
                if s in gold_sched:
                    emit_gold(gold_sched[s])
                scan_step(s)
                if s == W - 1:
                    # l1-renormalize all columns; keep log r (used by chunk 0)
                    psR = psfin_pool.tile([1, COLS], F32, tag="psR")
                    nc.tensor.matmul(psR[:], ones_k48[:], state[:],
                                     start=True, stop=True)
                    nc.scalar.activation(logr[:], psR[:],
                                         mybir.ActivationFunctionType.Ln)
                    nc.vector.reciprocal(rinv[:], psR[:])
                    psB = psfin_pool.tile([NL, COLS], F32, tag="psB")
                    nc.tensor.matmul(psB[:], ones_m48[:], rinv[:],
                                     start=True, stop=True)
                    nc.vector.tensor_tensor(state[:], psB[:], state[:],
                                            mybir.AluOpType.mult)

            # ---- finals ----
            psF0 = psfin_pool.tile([1, COLS], F32, tag="psF0")
            nc.tensor.matmul(psF0[:], ones_k48[:], state[:], start=True,
                             stop=True)
            nc.scalar.activation(lw_ones[:], psF0[:],
                                 mybir.ActivationFunctionType.Ln)
            psF1 = psfin_pool.tile([1, COLS], F32, tag="psF1")
            nc.tensor.matmul(psF1[:], expEnd_sb[:], state[:], start=True,
                             stop=True)
            nc.scalar.activation(lw_end[:], psF1[:],
                                 mybir.ActivationFunctionType.Ln)

            nc.sync.dma_start(out_scan[0:1, :], lw_ones[:])
            nc.sync.dma_start(out_scan[1:2, :], lw_end[:])
            nc.sync.dma_start(out_scan[2:3, :], logr[:])
            nc.sync.dma_start(out_gold[:], emitg[:])

    nc.finalize()
    _prog_cache["nc"] = nc
    return nc


def kernel(emissions, labels, mask, transitions, start_transitions,
           end_transitions, _results_hook=None):
    emissions = np.asarray(emissions, dtype=np.float32)
    labels = np.asarray(labels, dtype=np.int32)
    mask = np.asarray(mask)
    transitions = np.asarray(transitions, dtype=np.float32)
    start_transitions = np.asarray(start_transitions, dtype=np.float32)
    end_transitions = np.asarray(end_transitions, dtype=np.float32)
    assert mask.all(), "kernel specialized for the all-ones mask of this problem"

    nc = _build_program()

    expT_np = np.exp(transitions - CABS).astype(ml_dtypes.bfloat16)
    expStart_np = np.exp(start_transitions).reshape(NL, 1).astype(np.float32)
    expEnd_np = np.exp(end_transitions).reshape(NL, 1).astype(ml_dtypes.bfloat16)

    in_maps = []
    for k in range(NCORE):
        sl = slice(k * BLOC, (k + 1) * BLOC)
        in_maps.append({
            "emissions": np.ascontiguousarray(emissions[sl]),
            "labels": np.ascontiguousarray(labels[sl]),
            "exp_trans": expT_np,
            "exp_start": expStart_np,
            "exp_end": expEnd_np,
        })

    res = run_bass_kernel_spmd(nc, in_maps, core_ids=list(range(NCORE)))
    if _results_hook is not None:
        _results_hook(res)

    # ---- host-side unshard + tiny labels-only terms ----
    fwd = np.empty(B, dtype=np.float64)
    gold = np.empty(B, dtype=np.float64)
    tr_term = transitions[labels[:, 1:], labels[:, :-1]].sum(axis=1,
                                                            dtype=np.float64)
    st_term = start_transitions[labels[:, 0]].astype(np.float64)
    en_term = end_transitions[labels[:, -1]].astype(np.float64)

    for k in range(NCORE):
        o = res.results[k]
        lw_ones = o["out_scan"][0].astype(np.float64)   # [512] cols
        lw_end = o["out_scan"][1].astype(np.float64)
        logr = o["out_scan"][2].astype(np.float64)
        gold_dev = o["out_gold"].astype(np.float64)     # [128, 4]
        sl = slice(k * BLOC, (k + 1) * BLOC)

        cols = lw_ones.reshape(C, BLOC)
        cols_end = lw_end.reshape(C, BLOC)
        f = logr.reshape(C, BLOC)[0]  # chunk-0 columns carry the renorm scale
        f = f + cols[0:C - 1].sum(axis=0) + cols_end[C - 1]
        fwd[sl] = f + (T - 1) * CABS

        eg = gold_dev.sum(axis=1)  # [128] = per (b, half) partial sums
        gold[sl] = eg[:BLOC] + eg[BLOC:]

    gold += tr_term + st_term + en_term
    return np.float32(np.mean(fwd - gold))


if __name__ == "__main__":
    data = dict(np.load("/root/problem/inputs_cache.npz"))
    print(kernel(**data))
